# revision 12
# baseline (speedup 1.0000x reference)
"""DeltaNet forward kernel for Trainium2, sharded over 8 NeuronCores.

Sharding: core c handles batch c//2 and head-pair c%2 (heads {2*(c%2), 2*(c%2)+1}).
Host pre-transposes all weights/activations into the layouts the device needs
(hsT padded for the causal conv, per-head-pair weight slices pre-transposed and
pre-scaled, conv weights expanded to diagonal matmul operands), so the device
does no weight transposes. Projections run in bf16 (inputs rounded on host),
the delta-rule chunk math in f32r, Neumann internals in bf16.

Single ACT table set (silu_and_others): sigmoid via 0.5+0.5*tanh(x/2), all
rsqrt via a DVE bit-trick (int shift magic + 2 Newton steps) -- no Sqrt, no
Sigmoid table loads.

Per chunk the work is split into three pipeline stages so the PE always has
S-independent work: stage_a (projections + k-norm + Tinv/W/Mqk), s_advance
(the sequential U/O/S chain), tail (gated rmsnorm + output projection), with
stage_a running 2 chunks ahead of tail.
"""

import sys

for _p in ("/opt/trn_rl_repo", "/root/.axon_site"):
    if _p not in sys.path:
        sys.path.insert(0, _p)

import numpy as np
import ml_dtypes

import concourse.bass as bass
import concourse.tile as tile
from concourse import bacc, mybir
from concourse.bass_utils import run_bass_kernel_spmd
from concourse.masks import make_identity

F32 = mybir.dt.float32
F32R = mybir.dt.float32r
BF16 = mybir.dt.bfloat16
I32 = mybir.dt.int32
NPBF = ml_dtypes.bfloat16

B, L, D, H = 4, 2048, 1024, 4
DK, DV = 512, 1024
HK, HV = 128, 256
CONV, EPS = 4, 1e-5
C = 128            # delta-rule chunk length
NCH = L // C       # 16 chunks
LB = 512           # L-block for conv / q projection
CPB = LB // C      # 4 chunks per L-block
KD = D // 128      # 8 contraction slices
HPC = 2            # heads per core
N_CORES = 8
QSCALE = HK ** -0.5
WRC = HPC * HV + HPC * HV + HPC * HK + HPC   # 1282 row-proj cols: [v|g|k|beta]
MAGIC = 0x5F3759DF


def _mm(nc, out, lhsT, rhs, start, stop):
    """float32r matmul (full-rate 1 cycle/row)."""
    assert lhsT.dtype == F32R and rhs.dtype == F32R, (lhsT.dtype, rhs.dtype)
    nc.tensor.matmul(out, lhsT, rhs, start=start, stop=stop)


def build_program():
    nc = bacc.Bacc(
        "TRN2", target_bir_lowering=False, debug=False,
        enable_asserts=False, num_devices=N_CORES,
    )

    hsT = nc.dram_tensor("hsT", [D, L + 3], BF16, kind="ExternalInput").ap()
    wq = nc.dram_tensor("wq", [D, HPC * HK], BF16, kind="ExternalInput").ap()
    wr = nc.dram_tensor("wr", [D, WRC], BF16, kind="ExternalInput").ap()
    wo = nc.dram_tensor("wo", [HPC * HV, D], BF16, kind="ExternalInput").ap()
    dgd = nc.dram_tensor("dgd", [128, KD * CONV * 128], BF16, kind="ExternalInput").ap()
    y = nc.dram_tensor("y", [L, D], F32, kind="ExternalOutput").ap()

    with tile.TileContext(nc) as tc:
        _build_body(nc, tc, hsT, wq, wr, wo, dgd, y)
    nc.compile()
    return nc


def _build_body(nc, tc, hsT, wq, wr, wo, dgd, y):
    from contextlib import ExitStack

    AF = mybir.AluOpType
    ACT = mybir.ActivationFunctionType

    ctx = ExitStack()
    const = ctx.enter_context(tc.tile_pool(name="const", bufs=1))
    # PSUM: 8 banks split by pipeline stage so next-chunk projections never
    # wait behind the current chunk's serial Neumann/S chain.
    psP = ctx.enter_context(tc.tile_pool(name="psP", bufs=2, space="PSUM"))
    psC = ctx.enter_context(tc.tile_pool(name="psC", bufs=2, space="PSUM"))
    psS = ctx.enter_context(tc.tile_pool(name="psS", bufs=2, space="PSUM"))
    psT = ctx.enter_context(tc.tile_pool(name="psT", bufs=2, space="PSUM"))
    hpool = ctx.enter_context(tc.tile_pool(name="hpool", bufs=2))
    xpool = ctx.enter_context(tc.tile_pool(name="xpool", bufs=2))
    qk = ctx.enter_context(tc.tile_pool(name="qk", bufs=2))
    sS = ctx.enter_context(tc.tile_pool(name="sS", bufs=6))
    ck = ctx.enter_context(tc.tile_pool(name="ck", bufs=4))
    ckx = ctx.enter_context(tc.tile_pool(name="ckx", bufs=5))
    cv = ctx.enter_context(tc.tile_pool(name="cv", bufs=3))
    cu = ctx.enter_context(tc.tile_pool(name="cu", bufs=4))
    otp = ctx.enter_context(tc.tile_pool(name="otp", bufs=3))
    scr = ctx.enter_context(tc.tile_pool(name="scr", bufs=3))
    sm = ctx.enter_context(tc.tile_pool(name="sm", bufs=4))

    def cp_act(dst, src):
        nc.scalar.copy(dst, src)

    def cp_dve(dst, src):
        nc.vector.tensor_copy(dst, src)

    # alternating engine for the Neumann bf16 casts
    cp_state = [0]

    def cp_alt(dst, src):
        cp_state[0] ^= 1
        (cp_act if cp_state[0] else cp_dve)(dst, src)

    # ---- constants ----
    identf = const.tile([128, 128], F32)
    make_identity(nc, identf)
    identb = const.tile([128, 128], BF16)
    make_identity(nc, identb)
    # umask: 1 where free >= part (upper incl diag); lowm: 1 where free < part
    umask = const.tile([128, 128], F32)
    nc.gpsimd.memset(umask, 1.0)
    nc.gpsimd.affine_select(
        out=umask, in_=umask, compare_op=AF.is_ge, fill=0.0,
        base=0, channel_multiplier=-1, pattern=[[1, 128]],
    )
    lowm = const.tile([128, 128], F32)   # 1 where free < part  (= 1 - umask)
    nc.vector.tensor_scalar(lowm, umask, -1.0, 1.0, AF.mult, AF.add)
    magic = const.tile([128, 2], I32)
    nc.vector.memset(magic, MAGIC)
    ones_i = const.tile([128, 1], I32)
    nc.vector.memset(ones_i, 1)

    # ---- weights (pre-transposed on host; plain DMA) ----
    wqs = const.tile([128, KD, HPC * HK], BF16)
    wrs = const.tile([128, KD, WRC], BF16)
    for ks in range(KD):
        nc.sync.dma_start(out=wqs[:, ks, :], in_=wq[ks * 128:(ks + 1) * 128, :])
        nc.sync.dma_start(out=wrs[:, ks, :], in_=wr[ks * 128:(ks + 1) * 128, :])
    wos = const.tile([128, 4, D], BF16)
    for s in range(4):
        nc.sync.dma_start(out=wos[:, s, :], in_=wo[s * 128:(s + 1) * 128, :])
    dgs = const.tile([128, KD * CONV, 128], BF16)
    nc.sync.dma_start(
        out=dgs, in_=dgd.rearrange("p (t q) -> p t q", q=128)
    )

    # ---- state ----
    S = []
    for h in range(HPC):
        st = sS.tile([128, HV], F32R, tag="S")
        nc.vector.memset(st.bitcast(F32), 0.0)
        S.append(st)

    def rsqrt2(x, n):
        """1/sqrt(x) for x [128, n] f32 SBUF via int bit-trick + 2 Newtons."""
        sh = sm.tile([128, n], I32, tag="rs_sh")
        nc.vector.tensor_scalar(
            sh, x.bitcast(I32), ones_i[:, 0:1], None, AF.logical_shift_right
        )
        y0 = sm.tile([128, n], I32, tag="rs_y0")
        nc.vector.tensor_sub(y0, magic[:, 0:n], sh)
        yv = y0.bitcast(F32)
        for it in range(2):
            t = sm.tile([128, n], F32, tag=f"rs_t{it}")
            nc.vector.tensor_mul(t, yv, yv)
            a = sm.tile([128, n], F32, tag=f"rs_a{it}")
            nc.vector.scalar_tensor_tensor(
                out=a, in0=x, scalar=-0.5, in1=t, op0=AF.mult, op1=AF.mult
            )
            yn = sm.tile([128, n], F32, tag=f"rs_y{it}")
            nc.vector.scalar_tensor_tensor(
                out=yn, in0=a, scalar=1.5, in1=yv, op0=AF.add, op1=AF.mult
            )
            yv = yn
        return yv

    def stage_lb(lb):
        """Load hsT block, causal conv + silu -> xT, q projection -> qT."""
        hT = hpool.tile([128, KD, LB + 3], BF16, tag="hT")
        for ks in range(KD):
            nc.sync.dma_start(
                out=hT[:, ks, :],
                in_=hsT[ks * 128:(ks + 1) * 128, lb * LB:lb * LB + LB + 3],
            )
        xT = xpool.tile([128, KD, LB], BF16, tag="xT")
        for d in range(KD):
            pc = psP.tile([128, LB], F32, tag="psP")
            for j in range(CONV):
                nc.tensor.matmul(
                    pc, dgs[:, d * CONV + j, :], hT[:, d, j:j + LB],
                    start=(j == 0), stop=(j == CONV - 1),
                )
            nc.scalar.activation(xT[:, d, :], pc, ACT.Silu)
        qT = qk.tile([128, HPC, LB], F32R, tag="qT")
        for h in range(HPC):
            pp = psP.tile([128, LB], F32, tag="psP")
            for ks in range(KD):
                nc.tensor.matmul(
                    pp, wqs[:, ks, h * 128:(h + 1) * 128], xT[:, ks, :],
                    start=(ks == 0), stop=(ks == KD - 1),
                )
            (cp_act if h == 0 else cp_dve)(qT[:, h, :], pp)
        return qT, xT

    def stage_a(c, qT, xT):
        """S-independent chunk work: projections, k-norm, Tinv, W, Mqk."""
        ch = c % CPB
        csl = slice(ch * C, (ch + 1) * C)

        # k/beta projection first: its (serial) norm chain overlaps the
        # v/g projection matmuls that follow.
        pkb = psP.tile([128, HPC * HK + HPC], F32, tag="psP")
        for ks in range(KD):
            nc.tensor.matmul(pkb, xT[:, ks, csl], wrs[:, ks, 1024:WRC],
                             start=(ks == 0), stop=(ks == KD - 1))
        # beta = sigmoid(z) = 0.5 + 0.5*tanh(z/2); nbeta = -beta
        th = sm.tile([128, HPC], F32, tag="th")
        nc.scalar.activation(th, pkb[:, 256:258], ACT.Tanh, scale=0.5)
        beta = sm.tile([128, HPC], F32, tag="beta")
        nc.vector.tensor_scalar(beta, th, 0.5, 0.5, AF.mult, AF.add)
        nbeta = sm.tile([128, HPC], F32, tag="nbeta")
        nc.vector.tensor_scalar(nbeta, th, -0.5, -0.5, AF.mult, AF.add)
        # k norms (both heads batched into [128, 2])
        nsq = sm.tile([128, HPC], F32, tag="nsq")
        for h in range(HPC):
            sq = scr.tile([128, 128], F32, tag="sq")
            nc.scalar.activation(
                sq, pkb[:, h * 128:(h + 1) * 128], ACT.Square,
                accum_out=nsq[:, h:h + 1],
            )
        inv = rsqrt2(nsq, HPC)
        invc = sm.tile([128, HPC], F32, tag="invc")
        nc.vector.tensor_scalar_min(invc, inv, 1e6)

        pv = psP.tile([128, HPC * HV], F32, tag="psP")
        pg = psP.tile([128, HPC * HV], F32, tag="psP")
        for ks in range(KD):
            lx = xT[:, ks, csl]
            nc.tensor.matmul(pv, lx, wrs[:, ks, 0:512],
                             start=(ks == 0), stop=(ks == KD - 1))
            nc.tensor.matmul(pg, lx, wrs[:, ks, 512:1024],
                             start=(ks == 0), stop=(ks == KD - 1))
        sg = cv.tile([128, HPC * HV], BF16, tag="sg")
        nc.scalar.activation(sg, pg, ACT.Silu)
        vb = cv.tile([128, HPC * HV], F32R, tag="vb")
        nc.vector.tensor_scalar_mul(vb[:, 0:HV], pv[:, 0:HV], beta[:, 0:1])
        nc.vector.tensor_scalar_mul(vb[:, HV:2 * HV], pv[:, HV:2 * HV], beta[:, 1:2])

        art = {"vb": vb, "sg": sg, "qT": qT, "csl": csl, "h": []}
        for h in range(HPC):
            knr = ckx.tile([128, 128], F32R, tag="knr")
            nc.vector.tensor_scalar_mul(
                knr, pkb[:, h * 128:(h + 1) * 128], invc[:, h:h + 1]
            )
            kbr = ck.tile([128, 128], F32R, tag="kbr")   # -beta * kn rows
            nc.vector.tensor_scalar_mul(kbr, knr.bitcast(F32), nbeta[:, h:h + 1])
            ptk = psC.tile([128, 128], F32, tag="psC")
            nc.tensor.transpose(ptk, knr.bitcast(F32), identf)
            knT = ck.tile([128, 128], F32R, tag="knT")
            cp_act(knT, ptk)

            # G = Kn Kn^T; Nb = strict_lower(-beta_i G) = M^T; Mb = M
            pG = psC.tile([128, 128], F32, tag="psC")
            _mm(nc, pG, knT, knT, start=True, stop=True)
            Nb = ck.tile([128, 128], BF16, tag="Nb")
            nc.vector.scalar_tensor_tensor(
                out=Nb, in0=pG, scalar=nbeta[:, h:h + 1], in1=lowm,
                op0=AF.mult, op1=AF.mult,
            )
            ptm = psC.tile([128, 128], BF16, tag="psC")
            nc.tensor.transpose(ptm, Nb, identb)
            Mb = ck.tile([128, 128], BF16, tag="Mb")
            cp_act(Mb, ptm)

            # Mqk^T = masked Kn Q^T
            pM = psC.tile([128, 128], F32, tag="psC")
            _mm(nc, pM, knT, qT[:, h, csl], start=True, stop=True)
            mqk = ckx.tile([128, 128], F32R, tag="mqk")
            nc.vector.tensor_mul(mqk, pM, umask)

            # TinvT = sum_{k<8} M^k via bf16 doubling
            S2 = ck.tile([128, 128], BF16, tag="S2")
            nc.vector.tensor_add(S2, Mb, identb)

            def mmb(lhsT, rhs):
                po = psC.tile([128, 128], F32, tag="psC")
                nc.tensor.matmul(po, lhsT, rhs, start=True, stop=True)
                return po

            def cast_b(po, tag):
                t = ck.tile([128, 128], BF16, tag=tag)
                cp_alt(t, po)
                return t

            P2 = cast_b(mmb(Nb, Mb), "P2")     # M @ M
            P2T = cast_b(mmb(Mb, Nb), "P2T")   # (M @ M)^T
            S4 = ck.tile([128, 128], BF16, tag="S4")
            nc.vector.tensor_add(S4, S2, mmb(P2T, S2))
            P4T = cast_b(mmb(P2, P2T), "P4T")
            tinvT = ckx.tile([128, 128], F32R, tag="tinvT")
            nc.vector.tensor_add(tinvT, S4, mmb(P4T, S4))

            # -W^T = Kb'^T TinvT with Kb' = -beta*Kn (negative folded in kbr)
            pW = psC.tile([128, 128], F32, tag="psC")
            _mm(nc, pW, kbr, tinvT, start=True, stop=True)
            nWT = ckx.tile([128, 128], F32R, tag="nWT")
            cp_dve(nWT, pW)
            art["h"].append({"knr": knr, "mqk": mqk, "tinvT": tinvT, "nWT": nWT})
        return art

    def s_advance(c, art):
        """Sequential S-chain: U, O (matmuls only), S update."""
        vb, qT, csl = art["vb"], art["qT"], art["csl"]
        art["O"] = []
        for h in range(HPC):
            a = art["h"][h]
            hsl = slice(h * HV, (h + 1) * HV)
            pU = psS.tile([128, HV], F32, tag="psS")
            _mm(nc, pU, a["nWT"], S[h], start=True, stop=False)
            _mm(nc, pU, a["tinvT"], vb[:, hsl], start=False, stop=True)
            U = cu.tile([128, HV], F32R, tag="U")
            cp_dve(U, pU)

            pO = psS.tile([128, HV], F32, tag="psS")
            _mm(nc, pO, qT[:, h, csl], S[h], start=True, stop=False)
            _mm(nc, pO, a["mqk"], U, start=False, stop=True)
            O_s = cu.tile([128, HV], F32, tag="O")
            cp_act(O_s, pO)
            art["O"].append(O_s)

            pD = psS.tile([128, HV], F32, tag="psS")
            _mm(nc, pD, a["knr"], U, start=True, stop=True)
            Sn = sS.tile([128, HV], F32R, tag="S")
            nc.vector.tensor_add(Sn, S[h].bitcast(F32), pD)
            S[h] = Sn

    def tail(c, art):
        """Gated rmsnorm + output projection + store."""
        sg = art["sg"]
        ms = sm.tile([128, HPC], F32, tag="ms")
        for h in range(HPC):
            O_s = art["O"][h]
            sq2 = scr.tile([128, HV], F32, tag="sq2")
            nc.scalar.activation(sq2, O_s, ACT.Square, accum_out=ms[:, h:h + 1])
        msb = sm.tile([128, HPC], F32, tag="msb")
        nc.vector.tensor_scalar(msb, ms, 1.0 / HV, EPS, AF.mult, AF.add)
        rs = rsqrt2(msb, HPC)
        ofin = cv.tile([128, HPC * HV], F32, tag="ofin")
        for h in range(HPC):
            hsl = slice(h * HV, (h + 1) * HV)
            nc.vector.scalar_tensor_tensor(
                out=ofin[:, hsl], in0=art["O"][h], scalar=rs[:, h:h + 1],
                in1=sg[:, hsl], op0=AF.mult, op1=AF.mult,
            )
        oT = otp.tile([128, 4, 128], BF16, tag="oT")
        for s in range(4):
            pt = psT.tile([128, 128], F32, tag="psT")
            nc.tensor.transpose(pt, ofin[:, s * 128:(s + 1) * 128], identf)
            cp_dve(oT[:, s, :], pt)
        for t2 in range(2):
            py = psT.tile([128, 512], F32, tag="psT")
            for s in range(4):
                nc.tensor.matmul(
                    py, oT[:, s, :], wos[:, s, t2 * 512:(t2 + 1) * 512],
                    start=(s == 0), stop=(s == 3),
                )
            yst = cv.tile([128, 512], F32, tag="yst")
            cp_act(yst, py)
            nc.sync.dma_start(
                out=y[c * 128:(c + 1) * 128, t2 * 512:(t2 + 1) * 512], in_=yst
            )

    # software pipeline: stage_a(c) | s_advance(c-1) | tail(c-2)
    arts = {}
    cur = None
    for t in range(NCH + 2):
        if t < NCH:
            if t % CPB == 0:
                cur = stage_lb(t // CPB)
            arts[t] = stage_a(t, *cur)
        if 1 <= t <= NCH:
            s_advance(t - 1, arts[t - 1])
        if t >= 2:
            tail(t - 2, arts.pop(t - 2))

    ctx.close()


_nc_cache = None


def _get_nc():
    global _nc_cache
    if _nc_cache is None:
        _nc_cache = build_program()
    return _nc_cache


def make_in_maps(hidden_states, conv_w, Wq, Wk, Wv, Wb, Wg, Wo, rms_weight):
    f32 = lambda a: np.asarray(a, dtype=np.float32)
    hs, cw = f32(hidden_states), f32(conv_w)
    Wq, Wk, Wv, Wb, Wg, Wo, rmsw = (
        f32(Wq), f32(Wk), f32(Wv), f32(Wb), f32(Wg), f32(Wo), f32(rms_weight)
    )
    bf = lambda a: np.ascontiguousarray(a).astype(NPBF)

    # conv weights as diagonal matmul operands: dgd[p, (d*CONV+j)*128+q] = (p==q)*cw[d*128+p, j]
    dgd = np.zeros((128, KD * CONV * 128), np.float32)
    idx = np.arange(128)
    for d in range(KD):
        for j in range(CONV):
            dgd[idx, (d * CONV + j) * 128 + idx] = cw[d * 128:(d + 1) * 128, j]
    dgd = bf(dgd)

    rms2 = np.tile(rmsw, HPC)[:, None]  # [512, 1]
    in_maps = []
    for core in range(N_CORES):
        b, g = core // 2, core % 2
        hsT = np.zeros((D, L + 3), np.float32)
        hsT[:, 3:] = hs[b].T
        wrcat = np.concatenate(
            [
                Wv[g * HPC * HV:(g + 1) * HPC * HV].T,
                Wg[g * HPC * HV:(g + 1) * HPC * HV].T,
                Wk[g * HPC * HK:(g + 1) * HPC * HK].T,
                Wb[g * HPC:(g + 1) * HPC].T,
            ],
            axis=1,
        )  # [D, 1282]
        in_maps.append({
            "hsT": bf(hsT),
            "wq": bf(Wq[g * HPC * HK:(g + 1) * HPC * HK].T * QSCALE),
            "wr": bf(wrcat),
            "wo": bf(Wo[:, g * HPC * HV:(g + 1) * HPC * HV].T * rms2),
            "dgd": dgd,
        })
    return in_maps


def unshard(results):
    y = np.empty((B, L, D), np.float32)
    for b in range(B):
        y[b] = results[2 * b]["y"] + results[2 * b + 1]["y"]
    return y


def kernel(hidden_states, conv_w, Wq, Wk, Wv, Wb, Wg, Wo, rms_weight, **_ignored):
    nc = _get_nc()
    in_maps = make_in_maps(hidden_states, conv_w, Wq, Wk, Wv, Wb, Wg, Wo, rms_weight)
    res = run_bass_kernel_spmd(nc, in_maps, core_ids=list(range(N_CORES)))
    return unshard(res.results)
